# revision 2
# baseline (speedup 1.0000x reference)
"""Causal self-attention on 8 TRN2 NeuronCores — zero-collective design.

Sharding: batch x query-tile-parity mesh (4 x 2). Core c = 2b + s handles
batch b = c//2 and query tiles {s, s+2, ..., s+14} (1024 of 2048 rows),
ALL 16 heads, full D=1024 output columns. No cross-core communication:
each core redundantly computes K/V for all keys it needs (causal), which
costs ~1.4x FLOPs vs head-splitting but removes the ReduceScatter, its
barrier, and the DRAM bounce traffic entirely.

Everything runs in bf16 (f32 PSUM accumulation); the host pre-transposes
x and pre-packs all operands partition-major so every input lands in one
large contiguous DMA. Host reassembles the interleaved rows and upcasts.
"""

from contextlib import ExitStack

import numpy as np
import ml_dtypes

import concourse.bass as bass
import concourse.tile as tile
from concourse import bacc, mybir
from concourse.bass_utils import run_bass_kernel_spmd

F32 = mybir.dt.float32
BF16 = mybir.dt.bfloat16
AF = mybir.ActivationFunctionType

D = 1024          # model dim
T = 2048          # sequence length
B = 4             # batch
HD = 64           # head dim
NH = 16           # heads (all on every core)
DC = D // 128     # 8 contraction chunks
TT = T // 128     # 16 key tiles
QT = 8            # local query tiles per core
SCALE = 1.0 / 8.0  # 1/sqrt(HD)


def build(repeat=1):
    nc = bacc.Bacc("TRN2", target_bir_lowering=False, debug=False, num_devices=8)

    # all host-side arrays are packed partition-major: [128, chunk, free]
    xT_ext = nc.dram_tensor("xT", [128, DC, T], BF16, kind="ExternalInput").ap()
    xTq_ext = nc.dram_tensor("xTq", [128, DC, 1024], BF16, kind="ExternalInput").ap()
    wa_ext = nc.dram_tensor("wa", [128, DC, 3 * D], BF16, kind="ExternalInput").ap()
    wp_ext = nc.dram_tensor("wp", [128, DC, D], BF16, kind="ExternalInput").ap()
    mask_ext = nc.dram_tensor("masks", [128, TT, 128], BF16, kind="ExternalInput").ap()
    out_ext = nc.dram_tensor("out", [1024, D], BF16, kind="ExternalOutput").ap()

    with tile.TileContext(nc) as tc, ExitStack() as top:
        pers = top.enter_context(tc.tile_pool(name="pers", bufs=1))
        mask_sb = pers.tile([128, TT, 128], BF16, tag="masks")
        nc.sync.dma_start(mask_sb[:], mask_ext)

        def body(iv=None):
            with ExitStack() as ph23:
                p23 = ph23.enter_context(tc.tile_pool(name="p23", bufs=1))
                # outputs of phase 1, consumed by phases 2/3
                kT = [p23.tile([128, T], BF16, tag=f"kT{i}", name=f"kT{i}")
                      for i in range(DC)]
                qT = [p23.tile([128, 1024], BF16, tag=f"qT{i}", name=f"qT{i}")
                      for i in range(DC)]
                # v_sb[tt]: [128 keys, 16 heads, 64 v dims + 1 ones]
                v_sb = [p23.tile([128, NH, HD + 1], BF16, tag=f"v{i}", name=f"v{i}")
                        for i in range(TT)]
                yT = [p23.tile([128, 1024], BF16, tag=f"yT{i}", name=f"yT{i}")
                      for i in range(DC)]

                # ================= phase 1: QKV =================
                with ExitStack() as ph1:
                    p1 = ph1.enter_context(tc.tile_pool(name="p1", bufs=1))
                    qkvps = ph1.enter_context(
                        tc.tile_pool(name="qkvps", bufs=3, space="PSUM"))
                    cpool = ph1.enter_context(tc.tile_pool(name="cpool", bufs=2))

                    xT = p1.tile([128, DC, T], BF16, tag="xT")
                    xTq = p1.tile([128, DC, 1024], BF16, tag="xTq")
                    wa = p1.tile([128, DC, 3 * D], BF16, tag="wa")
                    nc.sync.dma_start(xT[:], xT_ext)
                    nc.sync.dma_start(xTq[:], xTq_ext)
                    nc.sync.dma_start(wa[:], wa_ext)

                    # k/q col-tiles per ct (2 heads each); early heads first
                    for ct in range(DC):
                        # kT[ct][:, :] = wa_k[:, ct].T @ xT  (keys: all T)
                        for tch in range(4):
                            ps = qkvps.tile([128, 512], F32, tag="ps")
                            for dc in range(DC):
                                nc.tensor.matmul(
                                    ps[:],
                                    wa[:, dc, D + ct * 128:D + (ct + 1) * 128],
                                    xT[:, dc, tch * 512:(tch + 1) * 512],
                                    start=(dc == 0), stop=(dc == DC - 1))
                            nc.any.tensor_copy(
                                kT[ct][:, tch * 512:(tch + 1) * 512], ps[:])
                        # qT[ct][:, :] = wa_q[:, ct].T @ xTq  (local q only)
                        for tch in range(2):
                            ps = qkvps.tile([128, 512], F32, tag="ps")
                            for dc in range(DC):
                                nc.tensor.matmul(
                                    ps[:],
                                    wa[:, dc, ct * 128:(ct + 1) * 128],
                                    xTq[:, dc, tch * 512:(tch + 1) * 512],
                                    start=(dc == 0), stop=(dc == DC - 1))
                            nc.any.tensor_copy(
                                qT[ct][:, tch * 512:(tch + 1) * 512], ps[:])

                    # v natural: v[tt] = x[tt] @ wa_v  (all T keys, all heads)
                    for tt in range(TT):
                        for half in range(2):
                            ps = qkvps.tile([128, 512], F32, tag="ps")
                            for dc in range(DC):
                                nc.tensor.matmul(
                                    ps[:],
                                    xT[:, dc, tt * 128:(tt + 1) * 128],
                                    wa[:, dc,
                                       2 * D + half * 512:2 * D + (half + 1) * 512],
                                    start=(dc == 0), stop=(dc == DC - 1))
                            nc.any.tensor_copy(
                                v_sb[tt][:, half * 8:(half + 1) * 8, 0:HD],
                                ps[:].rearrange("p (h d) -> p h d", h=8))
                        nc.vector.memset(v_sb[tt][:, :, HD:HD + 1], 1.0)

                # ================= phase 2: attention =================
                with ExitStack() as ph2:
                    p2 = ph2.enter_context(tc.tile_pool(name="p2", bufs=1))
                    sps = ph2.enter_context(
                        tc.tile_pool(name="sps", bufs=2, space="PSUM"))
                    yps = ph2.enter_context(
                        tc.tile_pool(name="yps", bufs=2, space="PSUM"))
                    rps = ph2.enter_context(
                        tc.tile_pool(name="rps", bufs=1, space="PSUM"))
                    ppool = ph2.enter_context(tc.tile_pool(name="ppool", bufs=3))
                    npool = ph2.enter_context(tc.tile_pool(name="npool", bufs=2))

                    # ones row on partition 64 (lhsT for the PE broadcast
                    # of the reciprocal row, which DVE writes to lane 64)
                    ones_sb = p2.tile([65, 64], BF16, tag="ones")
                    nc.vector.memset(ones_sb[64:65, :], 1.0)

                    for h in range(NH):
                        ct = h // 2
                        p0 = 64 * (h % 2)
                        kT_h = kT[ct][p0:p0 + 64, :]
                        qT_h = qT[ct][p0:p0 + 64, :]
                        y_ps = yps.tile([65, 1024], F32, tag="y")
                        for i in range(TT):
                            o_i = 128 * (i // 2)      # first local q col of block
                            w = 1024 - o_i
                            p_sb = ppool.tile([128, 1024], BF16, tag="p")
                            off = 0
                            while off < w:
                                n = min(512, w - off)
                                s_ps = sps.tile([128, 512], F32, tag="s")
                                nc.tensor.matmul(
                                    s_ps[:, 0:n],
                                    kT_h[:, 128 * i:128 * (i + 1)],
                                    qT_h[:, o_i + off:o_i + off + n],
                                    start=True, stop=True)
                                nc.scalar.activation(
                                    p_sb[:, off:off + n], s_ps[:, 0:n],
                                    AF.Exp, scale=SCALE)
                                off += n
                            # first 128-col group: tri / zero / ones per core
                            nc.vector.tensor_mul(
                                p_sb[:, 0:128], p_sb[:, 0:128], mask_sb[:, i, :])
                            # AV accumulate, pieces aligned to 512 PSUM banks
                            aoff = 0
                            while aoff < w:
                                n = min(512 - (o_i + aoff) % 512, w - aoff)
                                bank = (o_i + aoff) // 512
                                nc.tensor.matmul(
                                    y_ps[:, o_i + aoff:o_i + aoff + n],
                                    v_sb[i][:, h, :],
                                    p_sb[:, aoff:aoff + n],
                                    start=(i == 0),
                                    stop=(i == (7 if bank == 0 else 15)))
                                aoff += n
                        # normalize y by softmax denominator (row 64 of y_ps):
                        # recip on lane 64, then PE ones-matmul broadcasts it
                        # down to partitions 0..63
                        l_f32 = npool.tile([65, 1024], F32, tag="lf32")
                        nc.vector.reciprocal(l_f32[64:65, :], y_ps[64:65, :])
                        l_sb = npool.tile([65, 1024], BF16, tag="lsb")
                        nc.vector.tensor_copy(l_sb[64:65, :], l_f32[64:65, :])
                        r_ps = rps.tile([64, 1024], F32, tag="rps")
                        for rh in range(2):
                            nc.tensor.matmul(
                                r_ps[:, rh * 512:(rh + 1) * 512],
                                ones_sb[64:65, :],
                                l_sb[64:65, rh * 512:(rh + 1) * 512],
                                start=True, stop=True)
                        recip_b = npool.tile([64, 1024], F32, tag="recipb")
                        nc.any.tensor_copy(recip_b[:], r_ps[:])
                        if h % 2 == 0:
                            nc.vector.tensor_mul(
                                yT[ct][0:64, :], y_ps[0:64, :], recip_b[:])
                        else:
                            ytmp = npool.tile([64, 1024], BF16, tag="ytmp")
                            nc.vector.tensor_mul(
                                ytmp[:], y_ps[0:64, :], recip_b[:])
                            nc.sync.dma_start(yT[ct][64:128, :], ytmp[:])

                # ================= phase 3: proj =================
                with ExitStack() as ph3:
                    p3 = ph3.enter_context(tc.tile_pool(name="p3", bufs=1))
                    ops = ph3.enter_context(
                        tc.tile_pool(name="ops", bufs=4, space="PSUM"))
                    opool = ph3.enter_context(tc.tile_pool(name="opool", bufs=3))

                    wp = p3.tile([128, DC, D], BF16, tag="wp")
                    nc.sync.dma_start(wp[:], wp_ext)

                    for j in range(QT):
                        o_sb = opool.tile([128, D], BF16, tag="osb")
                        for half in range(2):
                            ps = ops.tile([128, 512], F32, tag="o")
                            for dc in range(DC):
                                nc.tensor.matmul(
                                    ps[:],
                                    yT[dc][:, j * 128:(j + 1) * 128],
                                    wp[:, dc, half * 512:(half + 1) * 512],
                                    start=(dc == 0), stop=(dc == DC - 1))
                            nc.any.tensor_copy(
                                o_sb[:, half * 512:(half + 1) * 512], ps[:])
                        nc.sync.dma_start(
                            out_ext[j * 128:(j + 1) * 128, :], o_sb[:])

        if repeat == 1:
            body()
        else:
            with tc.For_i(0, repeat, 1) as iv:
                body(iv)

    nc.compile()
    return nc


def make_in_maps(x, W_attn, W_proj):
    bf = ml_dtypes.bfloat16
    # weights: same packed arrays for every core
    wa = np.ascontiguousarray(
        W_attn.reshape(DC, 128, 3 * D).transpose(1, 0, 2)).astype(bf)
    wp = np.ascontiguousarray(
        W_proj.reshape(DC, 128, D).transpose(1, 0, 2)).astype(bf)
    tri = np.triu(np.ones((128, 128), dtype=np.float32))
    in_maps = []
    for c in range(8):
        b, s = c // 2, c % 2
        xb = x[b]                                    # [2048, 1024]
        xT = np.ascontiguousarray(
            xb.T.reshape(DC, 128, T).transpose(1, 0, 2)).astype(bf)
        xq = xb.reshape(TT, 128, D)[s::2].reshape(1024, D)  # local q rows
        xTq = np.ascontiguousarray(
            xq.T.reshape(DC, 128, 1024).transpose(1, 0, 2)).astype(bf)
        # first-col-group mask per key tile i: tri if diagonal, else
        # zero (keys after queries) or ones (keys before queries)
        masks = np.empty((TT, 128, 128), dtype=np.float32)
        for i in range(TT):
            if i % 2 == s:
                masks[i] = tri
            elif s == 0:
                masks[i] = 0.0
            else:
                masks[i] = 1.0
        masks_p = np.ascontiguousarray(masks.transpose(1, 0, 2)).astype(bf)
        in_maps.append({
            "xT": xT, "xTq": xTq, "wa": wa, "wp": wp, "masks": masks_p,
        })
    return in_maps


_NC_CACHE = {}


def kernel(x, W_attn, W_proj):
    x = np.asarray(x, dtype=np.float32)
    W_attn = np.asarray(W_attn, dtype=np.float32)
    W_proj = np.asarray(W_proj, dtype=np.float32)
    if "nc" not in _NC_CACHE:
        _NC_CACHE["nc"] = build()
    nc = _NC_CACHE["nc"]
    in_maps = make_in_maps(x, W_attn, W_proj)
    res = run_bass_kernel_spmd(nc, in_maps, list(range(8)))
    out = np.empty((B, TT, 128, D), dtype=np.float32)
    for c in range(8):
        b, s = c // 2, c % 2
        out[b, s::2] = res.results[c]["out"].astype(np.float32).reshape(QT, 128, D)
    return out.reshape(B, T, D)
